# revision 9
# baseline (speedup 1.0000x reference)
# Trainium2 Bass kernel for nn_Block_SA (dense_cnn self-attention block).
#
# Per-sample computation (C=64 channels, 64x64 spatial, N=4096 positions):
#   v   = relu(bn1(conv1x1(x)))                      # V for attention
#   s   = (x^T x) / sqrt(C)                          # [N, N] scores, Q=K=x
#   p   = softmax(s, axis=-1)
#   a   = V p^T  (a[d,n] = sum_m p[n,m] V[d,m])
#   z   = relu(bn2(depthwise3x3(a)))
#   out = bn3(conv1x1(z)) + x
#
# Distribution: batch B=8, one sample per NeuronCore (data parallel, no
# collectives). BN params are folded into conv weights on the host.
#
# On-chip algorithm (per core):
#   - Scores computed TRANSPOSED: sT[m, n] tiles via matmul(lhsT=x[:,mtile],
#     rhs=x[:,nchunk]); softmax's sum over m becomes a matmul reduction
#     (ones column in the V^T blocks). K=64 contraction row-packs two
#     m-tiles at a time with tile_position for ~2x PE throughput.
#   - x is cast to bf16 PRE-SCALED by sqrt(2^7*log2e/8) so the PSUM scores
#     arrive as 2^7*log2(e) * (s/8). That feeds BOTH exp consumers:
#       * ACT: activation(Exp, scale=1/184.665) -- exact exp at 1 elem/
#         cycle/partition. ACT alone would be the bottleneck (~134us for
#         16.8M exps), so...
#       * DVE: Schraudolph bit-trick exp: int16 = round(psum + 16250.24)
#         bitcast as bf16 IS exp(s/8) with ~2% sawtooth error (washes to
#         <1e-3 after softmax normalization; validated vs reference).
#         One tensor_scalar_add per group, int16 convert on write.
#     Split per chunk: ACT 18 tiles (6 groups of 3), DVE 14 tiles
#     (groups {1,3,5,7,10}), balancing both engines at ~10.5us/chunk.
#   - exp outputs land in a whole-chunk persistent E buffer (2 buffers,
#     chunk parity) so AV consumption is decoupled from group rotation.
#   - AV accumulates over 32 m-tiles into one PSUM bank (K=128 bf16
#     matmuls, ~215ns streaming each); denominator via the ones column.
#   - Normalization: fast custom-DVE reciprocal + partition_broadcast on
#     GpSimd + DVE multiply.
#   - Depthwise 3x3 runs on the PE as 9 diag-weight matmuls over shifted
#     2-D window views of y (PSUM accumulation); ACT applies bias+relu
#     (per-partition bias AP). This keeps the DVE free for exp work.
#   - conv3 + bias via augmented ones row; residual add fp32 on DVE.
#   - Score matmuls are emitted in strict even/odd pairs and AV groups are
#     only injected at pair boundaries, so row-packed pairs stay adjacent
#     in the PE queue.

import numpy as np

_EPS = 1e-5
_C = 64
_CP1 = 65
_N = 4096
_CH = 512          # free-dim chunk (one PSUM bank of fp32)
_NCH = _N // _CH   # 8 chunks
_MT = 128          # m-tile (partition dim of transposed score tiles)
_NMT = _N // _MT   # 32 m-tiles
_W = 64            # image width
_NCONST = 129 + 9 * 64  # w1aug | w3aug | b2p | diags

_LOG2E = 1.4426950408889634
_ACT_A = 128.0 * _LOG2E           # 184.6649652...
_XSCALE = float(np.sqrt(_ACT_A / 8.0))   # 4.80448...
_ACT_SCALE = 1.0 / _ACT_A
_SCH_BIAS = (127.0 - 0.045) * 128.0      # 16250.24

# group sizes and DVE-consumed groups per chunk
_GROUPS_R = [3] * 10 + [2]
_DVE_R = frozenset({1, 3, 5, 7, 10})        # 14 tiles on DVE
_GROUPS_0 = [1, 2] + [3] * 9 + [2]
_DVE_0 = frozenset({3, 5, 7, 9, 11})        # 14 tiles on DVE

_STATE = {}


def _build_program(reps=1):
    import concourse.bacc as bacc
    import concourse.tile as tile
    from concourse import mybir

    F32 = mybir.dt.float32
    BF16 = mybir.dt.bfloat16
    I16 = mybir.dt.int16
    AF = mybir.ActivationFunctionType
    ALU = mybir.AluOpType

    nc = bacc.Bacc(None)

    xd = nc.dram_tensor("x", [_C, _N], F32, kind="ExternalInput")
    # packed weights -> one DMA: cols 0:64 w1aug, 64:128 w3aug,
    # 128 b2p, 129:705 diag(w2p[:,k]) k=0..8
    cd = nc.dram_tensor("consts", [_CP1, _NCONST], F32, kind="ExternalInput")
    onesd = nc.dram_tensor("ones_bf", [1, _N], BF16, kind="ExternalInput")
    outd = nc.dram_tensor("out", [_C, _N], F32, kind="ExternalOutput")

    with tile.TileContext(nc) as tc:
        with (
            tc.tile_pool(name="persist", bufs=1) as pp,
            tc.tile_pool(name="small", bufs=2) as sp,
            tc.tile_pool(name="ps_pool", bufs=2, space="PSUM") as psp,
            tc.tile_pool(name="po_pool", bufs=1, space="PSUM") as pop,
            tc.tile_pool(name="aux_pool", bufs=1, space="PSUM") as auxp,
        ):
            def emit_all():
                # ---- input staging. x DMA'd once (fp32, kept for the
                # residual), cast to bf16 WITH the Schraudolph pre-scale,
                # duplicated to partitions 64:128 via SBUF-to-SBUF DMA.
                xo = pp.tile([_C, _N], F32, name="xo", tag="xo")
                xa = pp.tile([_CP1, _N], BF16, name="xa", tag="xa")
                xb2 = pp.tile([_MT, _N], BF16, name="xb2", tag="xb2")
                lo = 0
                for s, w in enumerate([512, 512, 1024, 1024, 1024]):
                    sl = slice(lo, lo + w)
                    lo += w
                    nc.sync.dma_start(xo[:, sl], xd[:, sl])
                    if s < 3:
                        nc.vector.tensor_scalar_mul(xa[0:_C, sl], xo[:, sl], _XSCALE)
                    else:
                        # ACT is idle during staging; GpSimd's software
                        # convert loop is ~13x slower than DVE here
                        nc.scalar.mul(xa[0:_C, sl], xo[:, sl], _XSCALE)
                    nc.sync.dma_start(xb2[_C:_MT, sl], xa[0:_C, sl])
                nc.sync.dma_start(xa[_C:_CP1, :], onesd[:])

                # PE p-state warm-up (tensor engine needs ~3.4us of activity
                # to reach full clock; it idles during startup DMA anyway)
                wu = pp.tile([_C, _CH], BF16, name="wu", tag="wu")
                nc.vector.memset(wu[:], 0.5)
                wps = auxp.tile([_MT, _CH], F32, name="wps", tag="aux")
                for _ in range(10):
                    nc.tensor.matmul(
                        wps[0:_C, :], lhsT=wu[:, 0:_C], rhs=wu[:],
                        start=True, stop=True,
                    )

                cs = pp.tile([_CP1, _NCONST], F32, name="cs", tag="cs")
                nc.scalar.dma_start(cs[:], cd[:])
                b2s = cs[0:_C, 128:129]

                w1b = pp.tile([_CP1, _C], BF16, name="w1b", tag="w1b")
                nc.vector.tensor_copy(w1b[:], cs[:, 0:64])
                w3b = pp.tile([_CP1, _C], BF16, name="w3b", tag="w3b")
                nc.vector.tensor_copy(w3b[:], cs[:, 64:128])
                dgb = pp.tile([_C, 9 * _C], BF16, name="dgb", tag="dgb")
                nc.gpsimd.tensor_copy(dgb[:], cs[0:_C, 129:129 + 9 * _C])

                # V^T blocks: per m-tile a [128, 65] block (col 64 = ones)
                vt = pp.tile([_MT, _NMT * _CP1], BF16, name="vt", tag="vt")
                vt3 = vt.rearrange("p (t c) -> p t c", c=_CP1)
                nc.gpsimd.memset(vt3[:, :, _C:_CP1], 1.0)

                # whole-chunk E buffers (32 tiles x 512 bf16), chunk parity
                ptb0 = pp.tile([_MT, _NMT * _CH], BF16, name="ptb0", tag="ptb0")
                ptb1 = pp.tile([_MT, _NMT * _CH], BF16, name="ptb1", tag="ptb1")
                pti0 = ptb0.bitcast(I16)
                pti1 = ptb1.bitcast(I16)

                # normalized attention output with zeroed pad row each side
                yrp = pp.tile([_C, _N + 2 * _W], BF16, name="yrp", tag="yrp")
                nc.gpsimd.memset(yrp[:, 0:_W], 0.0)
                nc.gpsimd.memset(yrp[:, _W + _N : _N + 2 * _W], 0.0)
                yr = yrp[:, _W : _W + _N]
                yrp3 = yrp.rearrange("c (h w) -> c h w", w=_W)  # row i = y row i-1
                # post-depthwise activations (+ones row) feeding conv3
                zr = pp.tile([_CP1, _N], BF16, name="zr", tag="zr")
                nc.sync.dma_start(zr[_C:_CP1, :], onesd[:])
                zrv = zr[0:_C, :].rearrange("c (h w) -> c h w", w=_W)

                # ---- V^T groups: emitted lazily inside chunk 0's loop.
                _vt_emitted = [0]

                def emit_vt_groups(need_mtiles):
                    while _vt_emitted[0] * 8 < need_mtiles:
                        g = _vt_emitted[0]
                        vps = auxp.tile([_MT, 8 * _C], F32, name="vps", tag="aux")
                        for j in range(8):
                            m = 8 * g + j
                            nc.tensor.matmul(
                                vps[:, _C * j : _C * (j + 1)],
                                lhsT=xa[:, _MT * m : _MT * (m + 1)],
                                rhs=w1b[:],
                                start=True,
                                stop=True,
                            )
                        nc.vector.tensor_relu(
                            vt3[:, 8 * g : 8 * (g + 1), 0:_C],
                            vps[:].rearrange("p (t c) -> p t c", c=_C),
                        )
                        _vt_emitted[0] += 1

                # ---- depthwise 3x3 on the PE: 9 diag-weight matmuls over
                # clipped 2-D window views, accumulated in an aux PSUM bank;
                # ACT applies per-channel bias + relu into zr.
                def emit_dw_taps(h0, h1):
                    # PE diag-matmul taps into an aux PSUM bank; returns the
                    # bank so the (deferred) relu can read it later
                    nh = h1 - h0
                    dwp = auxp.tile([_C, nh * _W], F32, name="dwp", tag="aux")
                    dwp3 = dwp.rearrange("c (h w) -> c h w", w=_W)
                    taps = []
                    for k in [4, 0, 1, 2, 3, 5, 6, 7, 8]:
                        dy, dx = k // 3 - 1, k % 3 - 1
                        hh0, hh1 = max(h0, -dy), min(h1, _W - dy)
                        if hh1 <= hh0:
                            continue
                        x0, x1 = max(0, -dx), _W - max(0, dx)
                        taps.append((k, hh0, hh1, x0, x1, dy, dx))
                    for i, (k, hh0, hh1, x0, x1, dy, dx) in enumerate(taps):
                        nc.tensor.matmul(
                            dwp3[:, hh0 - h0 : hh1 - h0, x0:x1],
                            lhsT=dgb[:, _C * k : _C * (k + 1)],
                            rhs=yrp3[:, hh0 + dy + 1 : hh1 + dy + 1,
                                     x0 + dx : x1 + dx],
                            start=(i == 0),
                            stop=(i == len(taps) - 1),
                            skip_group_check=True,
                        )
                    return dwp3

                def emit_dw_relu(dwp3, h0, h1):
                    # emitted 2 groups after the taps so the in-order ACT
                    # queue never blocks waiting on the PE
                    nc.scalar.activation(
                        zrv[:, h0:h1, :], dwp3[:], AF.Relu, bias=b2s, scale=1.0
                    )

                def emit_dw(h0, h1):
                    emit_dw_relu(emit_dw_taps(h0, h1), h0, h1)

                def emit_conv3(c):
                    # conv3 (+bias via ones row) + residual + store
                    pc = auxp.tile([_C, _CH], F32, name="pc", tag="aux")
                    nc.tensor.matmul(
                        pc[:],
                        lhsT=w3b[:],
                        rhs=zr[:, _CH * c : _CH * (c + 1)],
                        start=True,
                        stop=True,
                    )
                    outt = sp.tile([_C, _CH], F32, name="outt", tag="outt", bufs=2)
                    nc.vector.tensor_tensor(
                        outt[:], pc[:], xo[:, _CH * c : _CH * (c + 1)], op=ALU.add
                    )
                    nc.sync.dma_start(outd[:, _CH * c : _CH * (c + 1)], outt[:])

                # ---- main fused-attention loop over n-chunks ----
                pending = []
                av_q = []
                _AV_DELAY = 4

                def emit_normalize(po, ci):
                    dsb = sp.tile([1, _CH], F32, name="dsb", tag="dsb", bufs=2)
                    nc.vector.tensor_copy(dsb[:], po[_C : _C + 1, :])
                    invf = sp.tile([1, _CH], F32, name="invf", tag="invf", bufs=2)
                    nc.vector.reciprocal_approx_fast(out=invf[:], in_=dsb[:])
                    bcps = sp.tile([_C, _CH], F32, name="bcps", tag="bcps", bufs=2)
                    nc.gpsimd.partition_broadcast(bcps[:], invf[:])
                    nc.vector.tensor_tensor(
                        yr[:, _CH * ci : _CH * (ci + 1)], po[0:_C, :], bcps[:],
                        op=ALU.mult,
                    )
                    box = {}

                    def taps_main(ci=ci, box=box):
                        box["m"] = emit_dw_taps(8 * ci, 8 * ci + 7)
                    def relu_main(ci=ci, box=box):
                        emit_dw_relu(box["m"], 8 * ci, 8 * ci + 7)
                    pending.append(taps_main)
                    pending.append(relu_main)
                    if ci >= 1:
                        def taps_b(ci=ci, box=box):
                            box["b"] = emit_dw_taps(8 * ci - 1, 8 * ci)
                        def relu_b(ci=ci, box=box):
                            emit_dw_relu(box["b"], 8 * ci - 1, 8 * ci)
                        pending.append(taps_b)
                        pending.append(relu_b)
                        pending.append(lambda ci=ci: emit_conv3(ci - 1))

                def pop_av():
                    emit, need, fin_ci_po = av_q.pop(0)
                    if need is not None:
                        emit_vt_groups(need)
                    emit()
                    if fin_ci_po is not None:
                        emit_normalize(*fin_ci_po)

                for ci in range(_NCH):
                    po = pop.tile([_MT, _CH], F32, name="po", tag="po")
                    ptb = ptb0 if ci % 2 == 0 else ptb1
                    pti = pti0 if ci % 2 == 0 else pti1
                    groups = _GROUPS_0 if ci == 0 else _GROUPS_R
                    dve_set = _DVE_0 if ci == 0 else _DVE_R
                    m = 0
                    for gi, msz in enumerate(groups):
                        ps = psp.tile([_MT, _CH * msz], F32, name="ps", tag="ps")
                        for j in range(msz):
                            mt = m + j
                            if mt % 2 == 0:
                                src, rows, tp = xa, slice(0, _C), (0, 0)
                            else:
                                src, rows, tp = xb2, slice(_C, _MT), (_C, 0)
                            nc.tensor.matmul(
                                ps[:, _CH * j : _CH * (j + 1)],
                                lhsT=src[rows, _MT * mt : _MT * (mt + 1)],
                                rhs=src[rows, _CH * ci : _CH * (ci + 1)],
                                start=True,
                                stop=True,
                                tile_position=tp,
                            )
                            # inject AV work only at pair boundaries so
                            # row-packed score pairs stay adjacent
                            if mt % 2 == 1:
                                while len(av_q) > _AV_DELAY:
                                    pop_av()
                        sl = slice(_CH * m, _CH * (m + msz))
                        if gi in dve_set:
                            nc.vector.tensor_scalar_add(pti[:, sl], ps[:], _SCH_BIAS)
                        else:
                            nc.scalar.activation(
                                ptb[:, sl], ps[:], AF.Exp, scale=_ACT_SCALE
                            )

                        def av_group(po=po, ptb=ptb, m=m, msz=msz):
                            for j in range(msz):
                                mt = m + j
                                nc.tensor.matmul(
                                    po[0:_CP1, :],
                                    lhsT=vt[:, _CP1 * mt : _CP1 * (mt + 1)],
                                    rhs=ptb[:, _CH * mt : _CH * (mt + 1)],
                                    start=(mt == 0),
                                    stop=(mt == _NMT - 1),
                                    skip_group_check=True,
                                )

                        last = m + msz == _NMT
                        av_q.append((av_group, (m + msz) if ci == 0 else None,
                                     (po, ci) if last else None))
                        m += msz
                        if gi in (2, 4, 6, 8, 10) and pending:
                            pending.pop(0)()
                while av_q:
                    pop_av()
                for f in pending:
                    f()
                emit_dw(_N // _W - 1, _N // _W)  # last row (no dy=+1 tap)
                emit_conv3(_NCH - 1)

            if reps == 1:
                emit_all()
            else:
                with tc.For_i(0, reps, 1):
                    emit_all()

    nc.finalize()
    return nc


def _get_nc():
    if "nc" not in _STATE:
        _STATE["nc"] = _build_program()
    return _STATE["nc"]


def _prep_inputs(x, w1, bn1_g, bn1_b, bn1_m, bn1_v,
                 w2, bn2_g, bn2_b, bn2_m, bn2_v,
                 w3, bn3_g, bn3_b, bn3_m, bn3_v):
    f32 = np.float32
    x = np.asarray(x, f32)
    inv1 = np.asarray(bn1_g, f32) / np.sqrt(np.asarray(bn1_v, f32) + _EPS)
    w1p = np.asarray(w1, f32)[:, :, 0, 0] * inv1[:, None] / _XSCALE
    b1p = np.asarray(bn1_b, f32) - np.asarray(bn1_m, f32) * inv1
    w1aug = np.concatenate([w1p.T, b1p[None, :]], axis=0)

    inv2 = np.asarray(bn2_g, f32) / np.sqrt(np.asarray(bn2_v, f32) + _EPS)
    w2p = np.asarray(w2, f32)[:, 0].reshape(_C, 9) * inv2[:, None]
    b2p = (np.asarray(bn2_b, f32) - np.asarray(bn2_m, f32) * inv2)[:, None]

    inv3 = np.asarray(bn3_g, f32) / np.sqrt(np.asarray(bn3_v, f32) + _EPS)
    w3p = np.asarray(w3, f32)[:, :, 0, 0] * inv3[:, None]
    b3p = np.asarray(bn3_b, f32) - np.asarray(bn3_m, f32) * inv3
    w3aug = np.concatenate([w3p.T, b3p[None, :]], axis=0)

    consts = np.zeros((_CP1, _NCONST), f32)
    consts[:, 0:64] = w1aug
    consts[:, 64:128] = w3aug
    consts[0:_C, 128:129] = b2p
    for k in range(9):
        consts[0:_C, 129 + _C * k : 129 + _C * (k + 1)] = np.diag(w2p[:, k])

    import ml_dtypes
    ones_bf = np.ones((1, _N), dtype=ml_dtypes.bfloat16)
    B = x.shape[0]
    in_maps = []
    for i in range(B):
        in_maps.append({
            "x": np.ascontiguousarray(x[i].reshape(_C, _N)),
            "consts": consts,
            "ones_bf": ones_bf,
        })
    return in_maps


def kernel(**inputs) -> np.ndarray:
    from concourse.bass_utils import run_bass_kernel_spmd

    in_maps = _prep_inputs(**inputs)
    nc = _get_nc()
    _STATE["in_maps"] = in_maps
    res = run_bass_kernel_spmd(nc, in_maps, list(range(len(in_maps))))
    out = np.stack(
        [r["out"].reshape(_C, _W, _W) for r in res.results]
    ).astype(np.float32)
    return out


def profile_exec_time():
    """Re-run the last inputs with NTFF tracing; returns exec time in ns."""
    from concourse.bass_utils import run_bass_kernel_spmd

    nc = _get_nc()
    in_maps = _STATE.get("in_maps")
    assert in_maps is not None, "call kernel() first"
    res = run_bass_kernel_spmd(nc, in_maps, list(range(len(in_maps))), trace=True)
    return res


# revision 11
# speedup vs baseline: 1.1439x; 1.1439x over previous
# Trainium2 Bass kernel for nn_Block_SA (dense_cnn self-attention block).
#
# Per-sample computation (C=64 channels, 64x64 spatial, N=4096 positions):
#   v   = relu(bn1(conv1x1(x)))                      # V for attention
#   s   = (x^T x) / sqrt(C)                          # [N, N] scores, Q=K=x
#   p   = softmax(s, axis=-1)
#   a   = V p^T  (a[d,n] = sum_m p[n,m] V[d,m])
#   z   = relu(bn2(depthwise3x3(a)))
#   out = bn3(conv1x1(z)) + x
#
# Distribution: batch B=8, one sample per NeuronCore (data parallel, no
# collectives). BN params are folded into conv weights on the host.
#
# On-chip algorithm (per core):
#   - Scores computed TRANSPOSED: sT[m, n] tiles via matmul(lhsT=x[:,mtile],
#     rhs=x[:,nchunk]); softmax's sum over m becomes a matmul reduction
#     (ones column in the V^T blocks). K=64 contraction row-packs two
#     m-tiles at a time with tile_position for ~2x PE throughput.
#   - x is cast to bf16 PRE-SCALED by sqrt(2^7*log2e/8) so the PSUM scores
#     arrive as 2^7*log2(e) * (s/8). That feeds BOTH exp consumers:
#       * ACT: activation(Exp, scale=1/184.665) -- exact exp at 1 elem/
#         cycle/partition. ACT alone would be the bottleneck (~134us for
#         16.8M exps), so...
#       * DVE: Schraudolph bit-trick exp: int16 = round(psum + 16250.24)
#         bitcast as bf16 IS exp(s/8) with ~2% sawtooth error (washes to
#         <1e-3 after softmax normalization; validated vs reference).
#         One tensor_scalar_add per group, int16 convert on write.
#     Split per chunk: ACT 18 tiles (6 groups of 3), DVE 14 tiles
#     (groups {1,3,5,7,10}), balancing both engines at ~10.5us/chunk.
#   - exp outputs land in a whole-chunk persistent E buffer (2 buffers,
#     chunk parity) so AV consumption is decoupled from group rotation.
#   - AV accumulates over 32 m-tiles into one PSUM bank (K=128 bf16
#     matmuls, ~215ns streaming each); denominator via the ones column.
#   - Normalization: fast custom-DVE reciprocal + partition_broadcast on
#     GpSimd + DVE multiply.
#   - Depthwise 3x3 runs on the PE as 9 diag-weight matmuls over shifted
#     2-D window views of y (PSUM accumulation); ACT applies bias+relu
#     (per-partition bias AP). This keeps the DVE free for exp work.
#   - conv3 + bias via augmented ones row; residual add fp32 on DVE.
#   - Score matmuls are emitted in strict even/odd pairs and AV groups are
#     only injected at pair boundaries, so row-packed pairs stay adjacent
#     in the PE queue.

import numpy as np

_EPS = 1e-5
_C = 64
_CP1 = 65
_N = 4096
_CH = 512          # free-dim chunk (one PSUM bank of fp32)
_NCH = _N // _CH   # 8 chunks
_MT = 128          # m-tile (partition dim of transposed score tiles)
_NMT = _N // _MT   # 32 m-tiles
_W = 64            # image width
_NCONST = 129 + 9 * 64  # w1aug | w3aug | b2p | diags

_LOG2E = 1.4426950408889634
_ACT_A = 128.0 * _LOG2E           # 184.6649652...
_XSCALE = float(np.sqrt(_ACT_A / 8.0))   # 4.80448...
_ACT_SCALE = 1.0 / _ACT_A
_SCH_BIAS = (127.0 - 0.045) * 128.0      # 16250.24

# group sizes and DVE-consumed groups per chunk
_GROUPS_R = [3] * 10 + [2]
_DVE_R = frozenset({1, 3, 5, 7, 10})        # 14 tiles on DVE
_GROUPS_0 = [1, 2] + [3] * 9 + [2]
_DVE_0 = frozenset({3, 5, 7, 9, 11})        # 14 tiles on DVE

_STATE = {}


def _build_program(reps=1):
    import concourse.bacc as bacc
    import concourse.tile as tile
    from concourse import mybir

    F32 = mybir.dt.float32
    BF16 = mybir.dt.bfloat16
    I16 = mybir.dt.int16
    AF = mybir.ActivationFunctionType
    ALU = mybir.AluOpType

    nc = bacc.Bacc(None)

    xd = nc.dram_tensor("x", [_C, _N], F32, kind="ExternalInput")
    # packed weights -> one DMA: cols 0:64 w1aug, 64:128 w3aug,
    # 128 b2p, 129:705 diag(w2p[:,k]) k=0..8
    cd = nc.dram_tensor("consts", [_CP1, _NCONST], F32, kind="ExternalInput")
    onesd = nc.dram_tensor("ones_bf", [1, _N], BF16, kind="ExternalInput")
    outd = nc.dram_tensor("out", [_C, _N], F32, kind="ExternalOutput")

    with tile.TileContext(nc) as tc:
        with (
            tc.tile_pool(name="persist", bufs=1) as pp,
            tc.tile_pool(name="small", bufs=2) as sp,
            tc.tile_pool(name="ps_pool", bufs=2, space="PSUM") as psp,
            tc.tile_pool(name="po_pool", bufs=1, space="PSUM") as pop,
            tc.tile_pool(name="aux_pool", bufs=1, space="PSUM") as auxp,
        ):
            def emit_all():
                # ---- input staging. x DMA'd once (fp32, kept for the
                # residual), cast to bf16 WITH the Schraudolph pre-scale,
                # duplicated to partitions 64:128 via SBUF-to-SBUF DMA.
                xo = pp.tile([_C, _N], F32, name="xo", tag="xo")
                xa = pp.tile([_CP1, _N], BF16, name="xa", tag="xa")
                xb2 = pp.tile([_MT, _N], BF16, name="xb2", tag="xb2")
                lo = 0
                for s, w in enumerate([512, 512, 1024, 1024, 1024]):
                    sl = slice(lo, lo + w)
                    lo += w
                    nc.sync.dma_start(xo[:, sl], xd[:, sl])
                    if s < 3:
                        nc.vector.tensor_scalar_mul(xa[0:_C, sl], xo[:, sl], _XSCALE)
                    else:
                        # ACT is idle during staging; GpSimd's software
                        # convert loop is ~13x slower than DVE here
                        nc.scalar.mul(xa[0:_C, sl], xo[:, sl], _XSCALE)
                    nc.sync.dma_start(xb2[_C:_MT, sl], xa[0:_C, sl])
                nc.sync.dma_start(xa[_C:_CP1, :], onesd[:])

                # PE p-state warm-up (tensor engine needs ~3.4us of activity
                # to reach full clock; it idles during startup DMA anyway)
                wu = pp.tile([_C, _CH], BF16, name="wu", tag="wu")
                nc.vector.memset(wu[:], 0.5)
                wps = auxp.tile([_MT, _CH], F32, name="wps", tag="aux")
                for _ in range(10):
                    nc.tensor.matmul(
                        wps[0:_C, :], lhsT=wu[:, 0:_C], rhs=wu[:],
                        start=True, stop=True,
                    )

                cs = pp.tile([_CP1, _NCONST], F32, name="cs", tag="cs")
                nc.scalar.dma_start(cs[:], cd[:])
                b2s = cs[0:_C, 128:129]

                w1b = pp.tile([_CP1, _C], BF16, name="w1b", tag="w1b")
                nc.vector.tensor_copy(w1b[:], cs[:, 0:64])
                w3b = pp.tile([_CP1, _C], BF16, name="w3b", tag="w3b")
                nc.vector.tensor_copy(w3b[:], cs[:, 64:128])
                dgb = pp.tile([_C, 9 * _C], BF16, name="dgb", tag="dgb")
                nc.gpsimd.tensor_copy(dgb[:], cs[0:_C, 129:129 + 9 * _C])

                # V^T blocks: per m-tile a [128, 65] block (col 64 = ones)
                vt = pp.tile([_MT, _NMT * _CP1], BF16, name="vt", tag="vt")
                vt3 = vt.rearrange("p (t c) -> p t c", c=_CP1)
                nc.gpsimd.memset(vt3[:, :, _C:_CP1], 1.0)

                # whole-chunk E buffers (32 tiles x 512 bf16), chunk parity
                ptb0 = pp.tile([_MT, _NMT * _CH], BF16, name="ptb0", tag="ptb0")
                ptb1 = pp.tile([_MT, _NMT * _CH], BF16, name="ptb1", tag="ptb1")
                pti0 = ptb0.bitcast(I16)
                pti1 = ptb1.bitcast(I16)

                # normalized attention output with zeroed pad row each side
                yrp = pp.tile([_C, _N + 2 * _W], BF16, name="yrp", tag="yrp")
                nc.gpsimd.memset(yrp[:, 0:_W], 0.0)
                nc.gpsimd.memset(yrp[:, _W + _N : _N + 2 * _W], 0.0)
                yr = yrp[:, _W : _W + _N]
                yrp3 = yrp.rearrange("c (h w) -> c h w", w=_W)  # row i = y row i-1
                # post-depthwise activations (+ones row) feeding conv3
                zr = pp.tile([_CP1, _N], BF16, name="zr", tag="zr")
                nc.sync.dma_start(zr[_C:_CP1, :], onesd[:])
                zrv = zr[0:_C, :].rearrange("c (h w) -> c h w", w=_W)

                # ---- V^T groups: emitted lazily inside chunk 0's loop.
                _vt_emitted = [0]

                def emit_vt_groups(need_mtiles):
                    while _vt_emitted[0] * 8 < need_mtiles:
                        g = _vt_emitted[0]
                        vps = auxp.tile([_MT, 8 * _C], F32, name="vps", tag="aux")
                        for j in range(8):
                            m = 8 * g + j
                            nc.tensor.matmul(
                                vps[:, _C * j : _C * (j + 1)],
                                lhsT=xa[:, _MT * m : _MT * (m + 1)],
                                rhs=w1b[:],
                                start=True,
                                stop=True,
                            )
                        nc.vector.tensor_relu(
                            vt3[:, 8 * g : 8 * (g + 1), 0:_C],
                            vps[:].rearrange("p (t c) -> p t c", c=_C),
                        )
                        _vt_emitted[0] += 1

                # ---- depthwise 3x3 on the PE: 9 diag-weight matmuls over
                # clipped 2-D window views, accumulated in an aux PSUM bank;
                # ACT applies per-channel bias + relu into zr.
                def emit_dw_taps(h0, h1):
                    # PE diag-matmul taps into an aux PSUM bank; returns the
                    # bank so the (deferred) relu can read it later
                    nh = h1 - h0
                    dwp = auxp.tile([_C, nh * _W], F32, name="dwp", tag="aux")
                    dwp3 = dwp.rearrange("c (h w) -> c h w", w=_W)
                    taps = []
                    for k in [4, 0, 1, 2, 3, 5, 6, 7, 8]:
                        dy, dx = k // 3 - 1, k % 3 - 1
                        hh0, hh1 = max(h0, -dy), min(h1, _W - dy)
                        if hh1 <= hh0:
                            continue
                        x0, x1 = max(0, -dx), _W - max(0, dx)
                        taps.append((k, hh0, hh1, x0, x1, dy, dx))
                    for i, (k, hh0, hh1, x0, x1, dy, dx) in enumerate(taps):
                        nc.tensor.matmul(
                            dwp3[:, hh0 - h0 : hh1 - h0, x0:x1],
                            lhsT=dgb[:, _C * k : _C * (k + 1)],
                            rhs=yrp3[:, hh0 + dy + 1 : hh1 + dy + 1,
                                     x0 + dx : x1 + dx],
                            start=(i == 0),
                            stop=(i == len(taps) - 1),
                            skip_group_check=True,
                        )
                    return dwp3

                def emit_dw_relu(dwp3, h0, h1):
                    # emitted 2 groups after the taps so the in-order ACT
                    # queue never blocks waiting on the PE
                    nc.scalar.activation(
                        zrv[:, h0:h1, :], dwp3[:], AF.Relu, bias=b2s, scale=1.0
                    )

                def emit_dw(h0, h1):
                    emit_dw_relu(emit_dw_taps(h0, h1), h0, h1)

                def emit_conv3(c):
                    # conv3 (+bias via ones row) + residual + store
                    pc = auxp.tile([_C, _CH], F32, name="pc", tag="aux")
                    nc.tensor.matmul(
                        pc[:],
                        lhsT=w3b[:],
                        rhs=zr[:, _CH * c : _CH * (c + 1)],
                        start=True,
                        stop=True,
                    )
                    outt = sp.tile([_C, _CH], F32, name="outt", tag="outt", bufs=2)
                    nc.vector.tensor_tensor(
                        outt[:], pc[:], xo[:, _CH * c : _CH * (c + 1)], op=ALU.add
                    )
                    nc.sync.dma_start(outd[:, _CH * c : _CH * (c + 1)], outt[:])

                # ---- main fused-attention loop over n-chunks ----
                pending = []
                av_q = []
                _AV_DELAY = 4

                def emit_normalize(po, ci):
                    dsb = sp.tile([1, _CH], F32, name="dsb", tag="dsb", bufs=2)
                    nc.vector.tensor_copy(dsb[:], po[_C : _C + 1, :])
                    invf = sp.tile([1, _CH], F32, name="invf", tag="invf", bufs=2)
                    nc.vector.reciprocal_approx_fast(out=invf[:], in_=dsb[:])
                    bcps = sp.tile([_C, _CH], F32, name="bcps", tag="bcps", bufs=2)
                    nc.gpsimd.partition_broadcast(bcps[:], invf[:])
                    nc.vector.tensor_tensor(
                        yr[:, _CH * ci : _CH * (ci + 1)], po[0:_C, :], bcps[:],
                        op=ALU.mult,
                    )
                    # depthwise for chunk ci-1 runs now (it needed this
                    # chunk's first y row for its last row's dy=+1 tap);
                    # full 8-row blocks, image edges handled by clipping
                    def queue_dw(c):
                        box = {}

                        def taps(c=c, box=box):
                            box["p"] = emit_dw_taps(8 * c, 8 * c + 8)
                        def relu(c=c, box=box):
                            emit_dw_relu(box["p"], 8 * c, 8 * c + 8)
                        pending.append(taps)
                        pending.append(relu)
                        pending.append(lambda c=c: emit_conv3(c))
                    if ci >= 1:
                        queue_dw(ci - 1)
                    if ci == _NCH - 1:
                        queue_dw(ci)

                def pop_av():
                    emit, need, fin_ci_po = av_q.pop(0)
                    if need is not None:
                        emit_vt_groups(need)
                    emit()
                    if fin_ci_po is not None:
                        emit_normalize(*fin_ci_po)

                for ci in range(_NCH):
                    po = pop.tile([_MT, _CH], F32, name="po", tag="po")
                    ptb = ptb0 if ci % 2 == 0 else ptb1
                    pti = pti0 if ci % 2 == 0 else pti1
                    groups = _GROUPS_0 if ci == 0 else _GROUPS_R
                    dve_set = _DVE_0 if ci == 0 else _DVE_R
                    m = 0
                    for gi, msz in enumerate(groups):
                        ps = psp.tile([_MT, _CH * msz], F32, name="ps", tag="ps")
                        for j in range(msz):
                            mt = m + j
                            if mt % 2 == 0:
                                src, rows, tp = xa, slice(0, _C), (0, 0)
                            else:
                                src, rows, tp = xb2, slice(_C, _MT), (_C, 0)
                            nc.tensor.matmul(
                                ps[:, _CH * j : _CH * (j + 1)],
                                lhsT=src[rows, _MT * mt : _MT * (mt + 1)],
                                rhs=src[rows, _CH * ci : _CH * (ci + 1)],
                                start=True,
                                stop=True,
                                tile_position=tp,
                            )
                            # inject AV work only at pair boundaries so
                            # row-packed score pairs stay adjacent
                            if mt % 2 == 1:
                                while len(av_q) > _AV_DELAY:
                                    pop_av()
                        sl = slice(_CH * m, _CH * (m + msz))
                        if gi in dve_set:
                            nc.vector.tensor_scalar_add(pti[:, sl], ps[:], _SCH_BIAS)
                        else:
                            nc.scalar.activation(
                                ptb[:, sl], ps[:], AF.Exp, scale=_ACT_SCALE
                            )

                        def av_group(po=po, ptb=ptb, m=m, msz=msz):
                            for j in range(msz):
                                mt = m + j
                                nc.tensor.matmul(
                                    po[0:_CP1, :],
                                    lhsT=vt[:, _CP1 * mt : _CP1 * (mt + 1)],
                                    rhs=ptb[:, _CH * mt : _CH * (mt + 1)],
                                    start=(mt == 0),
                                    stop=(mt == _NMT - 1),
                                    skip_group_check=True,
                                )

                        last = m + msz == _NMT
                        av_q.append((av_group, (m + msz) if ci == 0 else None,
                                     (po, ci) if last else None))
                        m += msz
                        if gi in (4, 6, 8) and pending:
                            pending.pop(0)()
                while av_q:
                    pop_av()
                for f in pending:
                    f()

            if reps == 1:
                emit_all()
            else:
                with tc.For_i(0, reps, 1):
                    emit_all()

    nc.finalize()
    return nc


def _get_nc():
    if "nc" not in _STATE:
        _STATE["nc"] = _build_program()
    return _STATE["nc"]


def _prep_inputs(x, w1, bn1_g, bn1_b, bn1_m, bn1_v,
                 w2, bn2_g, bn2_b, bn2_m, bn2_v,
                 w3, bn3_g, bn3_b, bn3_m, bn3_v):
    f32 = np.float32
    x = np.asarray(x, f32)
    inv1 = np.asarray(bn1_g, f32) / np.sqrt(np.asarray(bn1_v, f32) + _EPS)
    w1p = np.asarray(w1, f32)[:, :, 0, 0] * inv1[:, None] / _XSCALE
    b1p = np.asarray(bn1_b, f32) - np.asarray(bn1_m, f32) * inv1
    w1aug = np.concatenate([w1p.T, b1p[None, :]], axis=0)

    inv2 = np.asarray(bn2_g, f32) / np.sqrt(np.asarray(bn2_v, f32) + _EPS)
    w2p = np.asarray(w2, f32)[:, 0].reshape(_C, 9) * inv2[:, None]
    b2p = (np.asarray(bn2_b, f32) - np.asarray(bn2_m, f32) * inv2)[:, None]

    inv3 = np.asarray(bn3_g, f32) / np.sqrt(np.asarray(bn3_v, f32) + _EPS)
    w3p = np.asarray(w3, f32)[:, :, 0, 0] * inv3[:, None]
    b3p = np.asarray(bn3_b, f32) - np.asarray(bn3_m, f32) * inv3
    w3aug = np.concatenate([w3p.T, b3p[None, :]], axis=0)

    consts = np.zeros((_CP1, _NCONST), f32)
    consts[:, 0:64] = w1aug
    consts[:, 64:128] = w3aug
    consts[0:_C, 128:129] = b2p
    for k in range(9):
        consts[0:_C, 129 + _C * k : 129 + _C * (k + 1)] = np.diag(w2p[:, k])

    import ml_dtypes
    ones_bf = np.ones((1, _N), dtype=ml_dtypes.bfloat16)
    B = x.shape[0]
    in_maps = []
    for i in range(B):
        in_maps.append({
            "x": np.ascontiguousarray(x[i].reshape(_C, _N)),
            "consts": consts,
            "ones_bf": ones_bf,
        })
    return in_maps


def kernel(**inputs) -> np.ndarray:
    from concourse.bass_utils import run_bass_kernel_spmd

    in_maps = _prep_inputs(**inputs)
    nc = _get_nc()
    _STATE["in_maps"] = in_maps
    res = run_bass_kernel_spmd(nc, in_maps, list(range(len(in_maps))))
    out = np.stack(
        [r["out"].reshape(_C, _W, _W) for r in res.results]
    ).astype(np.float32)
    return out


def profile_exec_time():
    """Re-run the last inputs with NTFF tracing; returns exec time in ns."""
    from concourse.bass_utils import run_bass_kernel_spmd

    nc = _get_nc()
    in_maps = _STATE.get("in_maps")
    assert in_maps is not None, "call kernel() first"
    res = run_bass_kernel_spmd(nc, in_maps, list(range(len(in_maps))), trace=True)
    return res


# revision 16
# speedup vs baseline: 1.2742x; 1.1139x over previous
# Trainium2 Bass kernel for nn_Block_SA (dense_cnn self-attention block).
#
# Per-sample computation (C=64 channels, 64x64 spatial, N=4096 positions):
#   v   = relu(bn1(conv1x1(x)))                      # V for attention
#   s   = (x^T x) / sqrt(C)                          # [N, N] scores, Q=K=x
#   p   = softmax(s, axis=-1)
#   a   = V p^T  (a[d,n] = sum_m p[n,m] V[d,m])
#   z   = relu(bn2(depthwise3x3(a)))
#   out = bn3(conv1x1(z)) + x
#
# Distribution: batch B=8, one sample per NeuronCore (data parallel, no
# collectives). BN params are folded into conv weights on the host.
#
# On-chip algorithm (per core):
#   - Scores computed TRANSPOSED: sT[m, n] tiles via matmul(lhsT=x[:,mtile],
#     rhs=x[:,nchunk]); softmax's sum over m becomes a matmul reduction
#     (ones column in the V^T blocks). K=64 contraction row-packs two
#     m-tiles at a time with tile_position for ~2x PE throughput.
#   - x is cast to bf16 PRE-SCALED by sqrt(2^7*log2e/8) so the PSUM scores
#     arrive as 2^7*log2(e) * (s/8). That feeds BOTH exp consumers:
#       * ACT: activation(Exp, scale=1/184.665) -- exact exp at 1 elem/
#         cycle/partition. ACT alone would be the bottleneck (~134us for
#         16.8M exps), so...
#       * DVE: Schraudolph bit-trick exp: int16 = round(psum + 16250.24)
#         bitcast as bf16 IS exp(s/8) with ~2% sawtooth error (washes to
#         <1e-3 after softmax normalization; validated vs reference).
#         One tensor_scalar_add per group, int16 convert on write.
#     Split per chunk: ACT 18 tiles (6 groups of 3), DVE 14 tiles
#     (groups {1,3,5,7,10}), balancing both engines at ~10.5us/chunk.
#   - exp outputs land in a whole-chunk persistent E buffer (2 buffers,
#     chunk parity) so AV consumption is decoupled from group rotation.
#   - AV accumulates over 32 m-tiles into one PSUM bank (K=128 bf16
#     matmuls, ~215ns streaming each); denominator via the ones column.
#   - Normalization: fast custom-DVE reciprocal + partition_broadcast on
#     GpSimd + DVE multiply.
#   - Depthwise 3x3 runs on the PE as 9 diag-weight matmuls over shifted
#     2-D window views of y (PSUM accumulation); ACT applies bias+relu
#     (per-partition bias AP). This keeps the DVE free for exp work.
#   - conv3 + bias via augmented ones row; residual add fp32 on DVE.
#   - Score matmuls are emitted in strict even/odd pairs and AV groups are
#     only injected at pair boundaries, so row-packed pairs stay adjacent
#     in the PE queue.

import numpy as np

_EPS = 1e-5
_C = 64
_CP1 = 65
_N = 4096
_CH = 512          # free-dim chunk (one PSUM bank of fp32)
_NCH = _N // _CH   # 8 chunks
_MT = 128          # m-tile (partition dim of transposed score tiles)
_NMT = _N // _MT   # 32 m-tiles
_W = 64            # image width
_NCONST = 129 + 9 * 64  # w1aug | w3aug | b2p | diags

_LOG2E = 1.4426950408889634
_ACT_A = 128.0 * _LOG2E           # 184.6649652...
_XSCALE = float(np.sqrt(_ACT_A / 8.0))   # 4.80448...
_ACT_SCALE = 1.0 / _ACT_A
_SCH_BIAS = (127.0 - 0.045) * 128.0      # 16250.24

# group sizes alternate [2,3] so score groups rotate through TWO tag slots
# (2+3=5 PSUM banks total), freeing a bank to double-buffer the AV
# accumulator. 2-groups (even gi) mostly go to DVE, 3-groups to ACT.
_GROUPS_R = [2, 3] * 6 + [2]                    # 13 groups, 32 tiles
_DVE_R = frozenset({2, 4, 6, 8, 10, 12, 11})    # 15 tiles on DVE

_STATE = {}


def _build_program(reps=1):
    import concourse.bacc as bacc
    import concourse.tile as tile
    from concourse import mybir

    F32 = mybir.dt.float32
    BF16 = mybir.dt.bfloat16
    I16 = mybir.dt.int16
    AF = mybir.ActivationFunctionType
    ALU = mybir.AluOpType

    nc = bacc.Bacc(None)

    xd = nc.dram_tensor("x", [_C, _N], F32, kind="ExternalInput")
    # packed weights -> one DMA: cols 0:64 w1aug, 64:128 w3aug,
    # 128 b2p, 129:705 diag(w2p[:,k]) k=0..8
    cd = nc.dram_tensor("consts", [_CP1, _NCONST], F32, kind="ExternalInput")
    onesd = nc.dram_tensor("ones_bf", [1, _N], BF16, kind="ExternalInput")
    outd = nc.dram_tensor("out", [_C, _N], F32, kind="ExternalOutput")

    with tile.TileContext(nc) as tc:
        with (
            tc.tile_pool(name="persist", bufs=1) as pp,
            tc.tile_pool(name="small", bufs=2) as sp,
            tc.tile_pool(name="ps_pool", bufs=1, space="PSUM") as psp,
            tc.tile_pool(name="po_pool", bufs=2, space="PSUM") as pop,
            tc.tile_pool(name="aux_pool", bufs=1, space="PSUM") as auxp,
        ):
            def emit_all():
                # ---- input staging. x DMA'd once (fp32, kept for the
                # residual), cast to bf16 WITH the Schraudolph pre-scale,
                # duplicated to partitions 64:128 via SBUF-to-SBUF DMA.
                xo = pp.tile([_C, _N], F32, name="xo", tag="xo")
                xa = pp.tile([_CP1, _N], BF16, name="xa", tag="xa")
                xb2 = pp.tile([_MT, _N], BF16, name="xb2", tag="xb2")
                lo = 0
                for s, w in enumerate([512, 512, 1024, 1024, 1024]):
                    sl = slice(lo, lo + w)
                    lo += w
                    nc.sync.dma_start(xo[:, sl], xd[:, sl])
                    if s < 3:
                        nc.vector.tensor_scalar_mul(xa[0:_C, sl], xo[:, sl], _XSCALE)
                    else:
                        # ACT is idle during staging; GpSimd's software
                        # convert loop is ~13x slower than DVE here
                        nc.scalar.mul(xa[0:_C, sl], xo[:, sl], _XSCALE)
                    nc.sync.dma_start(xb2[_C:_MT, sl], xa[0:_C, sl])
                nc.sync.dma_start(xa[_C:_CP1, :], onesd[:])

                # PE p-state warm-up (tensor engine needs ~3.4us of activity
                # to reach full clock; it idles during startup DMA anyway)
                wu = pp.tile([_C, _CH], BF16, name="wu", tag="wu")
                nc.vector.memset(wu[:], 0.5)
                wps = auxp.tile([_MT, _CH], F32, name="wps", tag="aux")
                for _ in range(10):
                    nc.tensor.matmul(
                        wps[0:_C, :], lhsT=wu[:, 0:_C], rhs=wu[:],
                        start=True, stop=True,
                    )

                cs = pp.tile([_CP1, _NCONST], F32, name="cs", tag="cs")
                nc.scalar.dma_start(cs[:], cd[:])
                b2s = cs[0:_C, 128:129]

                w1b = pp.tile([_CP1, _C], BF16, name="w1b", tag="w1b")
                nc.vector.tensor_copy(w1b[:], cs[:, 0:64])
                w3b = pp.tile([_CP1, _C], BF16, name="w3b", tag="w3b")
                nc.vector.tensor_copy(w3b[:], cs[:, 64:128])
                dgb = pp.tile([_C, 9 * _C], BF16, name="dgb", tag="dgb")
                nc.gpsimd.tensor_copy(dgb[:], cs[0:_C, 129:129 + 9 * _C])

                # V^T blocks: per m-tile a [128, 65] block (col 64 = ones)
                vt = pp.tile([_MT, _NMT * _CP1], BF16, name="vt", tag="vt")
                vt3 = vt.rearrange("p (t c) -> p t c", c=_CP1)
                nc.gpsimd.memset(vt3[:, :, _C:_CP1], 1.0)

                # whole-chunk E buffers (32 tiles x 512 bf16), chunk parity
                ptb0 = pp.tile([_MT, _NMT * _CH], BF16, name="ptb0", tag="ptb0")
                ptb1 = pp.tile([_MT, _NMT * _CH], BF16, name="ptb1", tag="ptb1")
                pti0 = ptb0.bitcast(I16)
                pti1 = ptb1.bitcast(I16)

                # normalized attention output with zeroed pad row each side
                yrp = pp.tile([_C, _N + 2 * _W], BF16, name="yrp", tag="yrp")
                nc.gpsimd.memset(yrp[:, 0:_W], 0.0)
                nc.gpsimd.memset(yrp[:, _W + _N : _N + 2 * _W], 0.0)
                yr = yrp[:, _W : _W + _N]
                yrp3 = yrp.rearrange("c (h w) -> c h w", w=_W)  # row i = y row i-1
                # post-depthwise activations (+ones row) feeding conv3
                zr = pp.tile([_CP1, _N], BF16, name="zr", tag="zr")
                nc.sync.dma_start(zr[_C:_CP1, :], onesd[:])
                zrv = zr[0:_C, :].rearrange("c (h w) -> c h w", w=_W)

                # ---- V^T groups: emitted lazily inside chunk 0's loop.
                _vt_emitted = [0]

                def emit_vt_groups(need_mtiles):
                    while _vt_emitted[0] * 8 < need_mtiles:
                        g = _vt_emitted[0]
                        vps = auxp.tile([_MT, 8 * _C], F32, name="vps", tag="aux")
                        for j in range(8):
                            m = 8 * g + j
                            nc.tensor.matmul(
                                vps[:, _C * j : _C * (j + 1)],
                                lhsT=xa[:, _MT * m : _MT * (m + 1)],
                                rhs=w1b[:],
                                start=True,
                                stop=True,
                            )
                        nc.vector.tensor_relu(
                            vt3[:, 8 * g : 8 * (g + 1), 0:_C],
                            vps[:].rearrange("p (t c) -> p t c", c=_C),
                        )
                        _vt_emitted[0] += 1

                # ---- depthwise 3x3 on the PE: 9 diag-weight matmuls over
                # clipped 2-D window views, accumulated in an aux PSUM bank;
                # ACT applies per-channel bias + relu into zr.
                def emit_dw_taps(h0, h1):
                    # PE diag-matmul taps into an aux PSUM bank; returns the
                    # bank so the (deferred) relu can read it later
                    nh = h1 - h0
                    dwp = auxp.tile([_C, nh * _W], F32, name="dwp", tag="aux")
                    dwp3 = dwp.rearrange("c (h w) -> c h w", w=_W)
                    taps = []
                    for k in [4, 0, 1, 2, 3, 5, 6, 7, 8]:
                        dy, dx = k // 3 - 1, k % 3 - 1
                        hh0, hh1 = max(h0, -dy), min(h1, _W - dy)
                        if hh1 <= hh0:
                            continue
                        x0, x1 = max(0, -dx), _W - max(0, dx)
                        taps.append((k, hh0, hh1, x0, x1, dy, dx))
                    for i, (k, hh0, hh1, x0, x1, dy, dx) in enumerate(taps):
                        nc.tensor.matmul(
                            dwp3[:, hh0 - h0 : hh1 - h0, x0:x1],
                            lhsT=dgb[:, _C * k : _C * (k + 1)],
                            rhs=yrp3[:, hh0 + dy + 1 : hh1 + dy + 1,
                                     x0 + dx : x1 + dx],
                            start=(i == 0),
                            stop=(i == len(taps) - 1),
                            skip_group_check=True,
                        )
                    return dwp3

                def emit_dw_relu(dwp3, h0, h1):
                    # emitted 2 groups after the taps so the in-order ACT
                    # queue never blocks waiting on the PE
                    nc.scalar.activation(
                        zrv[:, h0:h1, :], dwp3[:], AF.Relu, bias=b2s, scale=1.0
                    )

                def emit_dw(h0, h1):
                    emit_dw_relu(emit_dw_taps(h0, h1), h0, h1)

                def emit_conv3(c):
                    # conv3 (+bias via ones row) + residual + store
                    pc = auxp.tile([_C, _CH], F32, name="pc", tag="aux")
                    nc.tensor.matmul(
                        pc[:],
                        lhsT=w3b[:],
                        rhs=zr[:, _CH * c : _CH * (c + 1)],
                        start=True,
                        stop=True,
                    )
                    outt = sp.tile([_C, _CH], F32, name="outt", tag="outt", bufs=2)
                    nc.vector.tensor_tensor(
                        outt[:], pc[:], xo[:, _CH * c : _CH * (c + 1)], op=ALU.add
                    )
                    nc.sync.dma_start(outd[:, _CH * c : _CH * (c + 1)], outt[:])

                # ---- main fused-attention loop over n-chunks ----
                pending = []
                av_q = []
                _AV_DELAY = 4

                def emit_normalize(po, ci):
                    # den row staged to partition 0 on ACT (closest to PSUM;
                    # keeps the DVE queue free for exp work)
                    dsb = sp.tile([1, _CH], F32, name="dsb", tag="dsb", bufs=2)
                    nc.scalar.copy(dsb[:], po[_C : _C + 1, :])
                    invf = sp.tile([1, _CH], F32, name="invf", tag="invf", bufs=2)
                    nc.vector.reciprocal_approx_fast(out=invf[:], in_=dsb[:])
                    bcps = sp.tile([_C, _CH], F32, name="bcps", tag="bcps", bufs=2)
                    nc.gpsimd.partition_broadcast(bcps[:], invf[:])
                    nc.vector.tensor_tensor(
                        yr[:, _CH * ci : _CH * (ci + 1)], po[0:_C, :], bcps[:],
                        op=ALU.mult,
                    )
                    # depthwise for chunk ci-1 runs now (it needed this
                    # chunk's first y row for its last row's dy=+1 tap);
                    # full 8-row blocks, image edges handled by clipping
                    def queue_dw(c):
                        box = {}

                        def taps(c=c, box=box):
                            box["p"] = emit_dw_taps(8 * c, 8 * c + 8)
                        def relu(c=c, box=box):
                            emit_dw_relu(box["p"], 8 * c, 8 * c + 8)
                        pending.append(taps)
                        pending.append(relu)
                        pending.append(lambda c=c: emit_conv3(c))
                    if ci >= 1:
                        queue_dw(ci - 1)
                    if ci == _NCH - 1:
                        queue_dw(ci)

                def pop_av():
                    emit, need, fin_ci_po = av_q.pop(0)
                    if need is not None:
                        emit_vt_groups(need)
                    emit()
                    if fin_ci_po is not None:
                        emit_normalize(*fin_ci_po)

                for ci in range(_NCH):
                    po = pop.tile([_MT, _CH], F32, name="po", tag="po")
                    ptb = ptb0 if ci % 2 == 0 else ptb1
                    pti = pti0 if ci % 2 == 0 else pti1
                    groups = _GROUPS_R
                    dve_set = _DVE_R
                    m = 0
                    for gi, msz in enumerate(groups):
                        ps = psp.tile([_MT, _CH * msz], F32, name="ps",
                                      tag=f"ps{msz}")
                        for j in range(msz):
                            mt = m + j
                            if mt % 2 == 0:
                                src, rows, tp = xa, slice(0, _C), (0, 0)
                            else:
                                src, rows, tp = xb2, slice(_C, _MT), (_C, 0)
                            nc.tensor.matmul(
                                ps[:, _CH * j : _CH * (j + 1)],
                                lhsT=src[rows, _MT * mt : _MT * (mt + 1)],
                                rhs=src[rows, _CH * ci : _CH * (ci + 1)],
                                start=True,
                                stop=True,
                                tile_position=tp,
                            )
                            # inject AV work only at pair boundaries so
                            # row-packed score pairs stay adjacent
                            if mt % 2 == 1:
                                while len(av_q) > _AV_DELAY:
                                    pop_av()
                        sl = slice(_CH * m, _CH * (m + msz))
                        if gi in dve_set:
                            nc.vector.tensor_scalar_add(pti[:, sl], ps[:], _SCH_BIAS)
                        else:
                            nc.scalar.activation(
                                ptb[:, sl], ps[:], AF.Exp, scale=_ACT_SCALE
                            )

                        def av_group(po=po, ptb=ptb, m=m, msz=msz):
                            for j in range(msz):
                                mt = m + j
                                nc.tensor.matmul(
                                    po[0:_CP1, :],
                                    lhsT=vt[:, _CP1 * mt : _CP1 * (mt + 1)],
                                    rhs=ptb[:, _CH * mt : _CH * (mt + 1)],
                                    start=(mt == 0),
                                    stop=(mt == _NMT - 1),
                                    skip_group_check=True,
                                )

                        last = m + msz == _NMT
                        av_q.append((av_group, (m + msz) if ci == 0 else None,
                                     (po, ci) if last else None))
                        m += msz
                        if gi in (5, 7, 9) and pending:
                            pending.pop(0)()
                while av_q:
                    pop_av()
                for f in pending:
                    f()

            if reps == 1:
                emit_all()
            else:
                with tc.For_i(0, reps, 1):
                    emit_all()

    nc.finalize()
    return nc


def _get_nc():
    if "nc" not in _STATE:
        _STATE["nc"] = _build_program()
    return _STATE["nc"]


def _prep_inputs(x, w1, bn1_g, bn1_b, bn1_m, bn1_v,
                 w2, bn2_g, bn2_b, bn2_m, bn2_v,
                 w3, bn3_g, bn3_b, bn3_m, bn3_v):
    f32 = np.float32
    x = np.asarray(x, f32)
    inv1 = np.asarray(bn1_g, f32) / np.sqrt(np.asarray(bn1_v, f32) + _EPS)
    w1p = np.asarray(w1, f32)[:, :, 0, 0] * inv1[:, None] / _XSCALE
    b1p = np.asarray(bn1_b, f32) - np.asarray(bn1_m, f32) * inv1
    w1aug = np.concatenate([w1p.T, b1p[None, :]], axis=0)

    inv2 = np.asarray(bn2_g, f32) / np.sqrt(np.asarray(bn2_v, f32) + _EPS)
    w2p = np.asarray(w2, f32)[:, 0].reshape(_C, 9) * inv2[:, None]
    b2p = (np.asarray(bn2_b, f32) - np.asarray(bn2_m, f32) * inv2)[:, None]

    inv3 = np.asarray(bn3_g, f32) / np.sqrt(np.asarray(bn3_v, f32) + _EPS)
    w3p = np.asarray(w3, f32)[:, :, 0, 0] * inv3[:, None]
    b3p = np.asarray(bn3_b, f32) - np.asarray(bn3_m, f32) * inv3
    w3aug = np.concatenate([w3p.T, b3p[None, :]], axis=0)

    consts = np.zeros((_CP1, _NCONST), f32)
    consts[:, 0:64] = w1aug
    consts[:, 64:128] = w3aug
    consts[0:_C, 128:129] = b2p
    for k in range(9):
        consts[0:_C, 129 + _C * k : 129 + _C * (k + 1)] = np.diag(w2p[:, k])

    import ml_dtypes
    ones_bf = np.ones((1, _N), dtype=ml_dtypes.bfloat16)
    B = x.shape[0]
    in_maps = []
    for i in range(B):
        in_maps.append({
            "x": np.ascontiguousarray(x[i].reshape(_C, _N)),
            "consts": consts,
            "ones_bf": ones_bf,
        })
    return in_maps


def kernel(**inputs) -> np.ndarray:
    from concourse.bass_utils import run_bass_kernel_spmd

    in_maps = _prep_inputs(**inputs)
    nc = _get_nc()
    _STATE["in_maps"] = in_maps
    res = run_bass_kernel_spmd(nc, in_maps, list(range(len(in_maps))))
    out = np.stack(
        [r["out"].reshape(_C, _W, _W) for r in res.results]
    ).astype(np.float32)
    return out


def profile_exec_time():
    """Re-run the last inputs with NTFF tracing; returns exec time in ns."""
    from concourse.bass_utils import run_bass_kernel_spmd

    nc = _get_nc()
    in_maps = _STATE.get("in_maps")
    assert in_maps is not None, "call kernel() first"
    res = run_bass_kernel_spmd(nc, in_maps, list(range(len(in_maps))), trace=True)
    return res
